# revision 6
# baseline (speedup 1.0000x reference)
"""Cross-attention (softmax over queries) on 8 Trainium2 NeuronCores.

Reference (per batch b):
    q = y @ Wq.T + bq            [N, H]
    k = x @ Wk.T + bk            [M, H]
    v = x @ Wv.T + bv            [M, D]
    dots = (q @ k.T) * H**-0.5   [N, M]
    attn = softmax(dots, axis=0) (over queries n, per key column m)
    out  = attn @ v              [N, D]

Sharding: data-parallel over batch B=8, one batch per core (SPMD).

Host prep: y/x are shipped pre-transposed ([C, N] / [C, M]) and pre-cast to
fp16 (the same rounding the previous in-flight DMA cast applied); weights are
pre-transposed/pre-scaled fp16. All matmuls run fp16 with fp32 PSUM.

Device algorithm (per core):
  A. DMA yT/xT quarters; project qT[h,n], kT[h,m] per quarter (q/k biases
     added by the ACT psum->sbuf copy, per-partition).
  C. per 128-row key chunk mc: dotsT[m,n] into two [128,1024] PSUM halves,
     column max (DVE), fused exp+rowsum on ACT into attnT fp16; then
     V-projection chunk, +bv via DVE broadcast-add in PSUM, single ACT
     copy applies the 1/colsum fold (scale=rsum per-partition) -> v fp16.
  D. out[n,d] = sum_m attnT[m,n] * v'[m,d]; dense 16-matmul PSUM chains.
"""

from contextlib import ExitStack

import numpy as np

import concourse.mybir as mybir
import concourse.tile as tile
from concourse import bacc
from concourse.bass_utils import run_bass_kernel_spmd

F32 = mybir.dt.float32
F16 = mybir.dt.float16
Exp = mybir.ActivationFunctionType.Exp
Copy = mybir.ActivationFunctionType.Copy
AX = mybir.AxisListType.X

B, N, M, C, H, D = 8, 2048, 2048, 1024, 512, 1024
P = 128
NT, MT, CCH, HC = N // P, M // P, C // P, H // P  # 16, 16, 8, 4
SCALE = (C // 2) ** -0.5

_CACHE = {}


def _build_nc():
    nc = bacc.Bacc("TRN2", target_bir_lowering=False, debug=False)

    yt_d = nc.dram_tensor("yt", [C, N], F16, kind="ExternalInput").ap()
    xt_d = nc.dram_tensor("xt", [C, M], F16, kind="ExternalInput").ap()
    wqt_d = nc.dram_tensor("wqt", [C, H], F16, kind="ExternalInput").ap()
    wkt_d = nc.dram_tensor("wkt", [C, H], F16, kind="ExternalInput").ap()
    wvt_d = nc.dram_tensor("wvt", [C, D], F16, kind="ExternalInput").ap()
    bq_d = nc.dram_tensor("bq", [H], F32, kind="ExternalInput").ap()
    bk_d = nc.dram_tensor("bk", [H], F32, kind="ExternalInput").ap()
    bv_d = nc.dram_tensor("bv", [D], F32, kind="ExternalInput").ap()
    out_d = nc.dram_tensor("out", [N, D], F32, kind="ExternalOutput").ap()

    yt_r = yt_d.rearrange("(o p) n -> p o n", p=P)  # [128, 8, 2048]
    xt_r = xt_d.rearrange("(o p) n -> p o n", p=P)
    out_r = out_d.rearrange("(t p) d -> p t d", p=P)

    with tile.TileContext(nc) as tc:
        with (
            tc.tile_pool(name="persist", bufs=1) as pers,
            tc.tile_pool(name="stats", bufs=1) as stats,
            tc.tile_pool(name="xT_pool", bufs=1) as xTp,
        ):
            qT = pers.tile([P, HC, N], F16, tag="qT")  # [h%128, h//128, n] 2MB
            kT = pers.tile([P, HC, M], F16, tag="kT")  # 2MB
            v = pers.tile([P, MT, D], F16, tag="v")  # [m%128, m//128, d] 4MB
            wv_sb = pers.tile([P, CCH, D], F16, tag="wv")  # 2MB
            bv_rep = pers.tile([P, D], F32, tag="bv_rep")  # 4KB/part

            sums = stats.tile([P, MT], F32, tag="sums")
            rsum = stats.tile([P, MT], F32, tag="rsum")
            bq_sb = stats.tile([P, HC], F32, tag="bq")  # [h%128, h//128]
            bk_sb = stats.tile([P, HC], F32, tag="bk")
            bv_sb = stats.tile([1, D], F32, tag="bv")
            warm = stats.tile([P, 512], F16, tag="warm")

            xT = xTp.tile([P, CCH, M], F16, tag="xT")  # alive through phase C

            # ---------- Phase A: load yT/xT + q/k projections ----------
            ps_stack = ExitStack()
            psPP = ps_stack.enter_context(
                tc.tile_pool(name="ps_pp", bufs=4, space="PSUM")
            )
            with (
                tc.tile_pool(name="yT_pool", bufs=1) as yTp,
                tc.tile_pool(name="w_pool", bufs=1) as wp,
                tc.tile_pool(name="ps_w", bufs=1, space="PSUM") as psW,
            ):
                wq_sb = wp.tile([P, CCH, H], F16, tag="wq")  # [c%128, c//128, h]
                wk_sb = wp.tile([P, CCH, H], F16, tag="wk")
                wq_r = wqt_d.rearrange("(o p) h -> p o h", p=P)
                # split the first weight load per-hc so the first projection
                # chain can start after only 256KB of weight traffic
                for hc in range(HC):
                    nc.sync.dma_start(
                        wq_sb[:, :, hc * P : (hc + 1) * P],
                        wq_r[:, :, hc * P : (hc + 1) * P],
                    )
                nc.sync.dma_start(wk_sb[:], wkt_d.rearrange("(o p) h -> p o h", p=P))
                nc.sync.dma_start(bq_sb[:], bq_d.rearrange("(o p) -> p o", p=P))
                nc.sync.dma_start(bk_sb[:], bk_d.rearrange("(o p) -> p o", p=P))
                nc.sync.dma_start(bv_sb[:], bv_d[None, :])

                yT = yTp.tile([P, CCH, N], F16, tag="yT")
                # issue all activation loads up front; they pipeline on the
                # gpsimd queue while projections consume earlier quarters
                nc.gpsimd.dma_start(yT[:, :, 0:256], yt_r[:, :, 0:256])
                nc.gpsimd.dma_start(yT[:, :, 256:512], yt_r[:, :, 256:512])
                for j in range(1, 4):
                    nc.gpsimd.dma_start(
                        yT[:, :, j * 512 : (j + 1) * 512],
                        yt_r[:, :, j * 512 : (j + 1) * 512],
                    )
                for j in range(4):
                    nc.gpsimd.dma_start(
                        xT[:, :, j * 512 : (j + 1) * 512],
                        xt_r[:, :, j * 512 : (j + 1) * 512],
                    )
                nc.gpsimd.partition_broadcast(bv_rep[:], bv_sb[:1, :])
                nc.sync.dma_start(wv_sb[:], wvt_d.rearrange("(o p) d -> p o d", p=P))

                # PE warm-up: ramp the clock on junk matmuls while DMAs land
                nc.vector.memset(warm[:], 0.0)
                pw = psW.tile([P, 512], F32, tag="pw")
                with nc.named_scope("A_warm"):
                    for i in range(20):
                        nc.tensor.matmul(
                            pw[:], warm[:, :P], warm[:], start=(i == 0),
                            stop=(i == 19),
                        )

                def project_j(dst, w_sb, b_sb, src_T, j, blocks=((0, 512),)):
                    # one 512-wide column block of a projection, all hc chunks
                    for lo, hi in blocks:
                        for hc in range(HC):
                            pp = psPP.tile([P, 512], F32, tag="pp")
                            for cc in range(CCH):
                                nc.tensor.matmul(
                                    pp[:, : hi - lo],
                                    w_sb[:, cc, hc * P : (hc + 1) * P],
                                    src_T[:, cc, j * 512 + lo : j * 512 + hi],
                                    start=(cc == 0),
                                    stop=(cc == CCH - 1),
                                )
                            # ACT copy: psum -> f16, + per-partition bias
                            nc.scalar.add(
                                dst[:, hc, j * 512 + lo : j * 512 + hi],
                                pp[:, : hi - lo],
                                b_sb[:, hc : hc + 1],
                            )

                with nc.named_scope("A_yq"):
                    project_j(qT, wq_sb, bq_sb, yT, 0, ((0, 256), (256, 512)))
                    for j in range(1, 4):
                        project_j(qT, wq_sb, bq_sb, yT, j)
                with nc.named_scope("A_xk"):
                    for j in range(4):
                        project_j(kT, wk_sb, bk_sb, xT, j)

            ps_stack.close()  # free phase-A psum pool before phase C

            # ---------- Phase C: dots/softmax then V-proj chunks ----------
            with (
                tc.tile_pool(name="late", bufs=1) as late,
                tc.tile_pool(name="sc", bufs=4) as sc,
            ):
                psC_stack = ExitStack()
                psC = psC_stack.enter_context(
                    tc.tile_pool(name="ps_c", bufs=1, space="PSUM")
                )
                psV = psC_stack.enter_context(
                    tc.tile_pool(name="ps_v", bufs=2, space="PSUM")
                )
                attnT = late.tile([P, MT, N], F16, tag="attnT")  # 8MB

                def dots_chunk(mc):
                    halves = []
                    for h in range(2):
                        pd = psC.tile([P, 1024], F32, tag=f"dots{h}")
                        for j2 in range(2):
                            j = h * 2 + j2
                            for hc in range(HC):
                                nc.tensor.matmul(
                                    pd[:, j2 * 512 : (j2 + 1) * 512],
                                    kT[:, hc, mc * P : (mc + 1) * P],
                                    qT[:, hc, j * 512 : (j + 1) * 512],
                                    start=(hc == 0),
                                    stop=(hc == HC - 1),
                                )
                        halves.append(pd)
                    pmax = sc.tile([P, 2], F32, tag="pmax")
                    for h in range(2):
                        nc.vector.reduce_max(
                            pmax[:, h : h + 1], halves[h][:], axis=AX
                        )
                    negmax = sc.tile([P, 1], F32, tag="negmax")
                    nc.vector.reduce_max(negmax[:], pmax[:], axis=AX, negate=True)
                    ssum = sc.tile([P, 2], F32, tag="ssum")
                    for h in range(2):
                        nc.scalar.activation(
                            out=attnT[:, mc, h * 1024 : (h + 1) * 1024],
                            in_=halves[h][:],
                            func=Exp,
                            bias=negmax[:],
                            accum_out=ssum[:, h : h + 1],
                        )
                    nc.vector.tensor_tensor(
                        sums[:, mc : mc + 1],
                        ssum[:, 0:1],
                        ssum[:, 1:2],
                        mybir.AluOpType.add,
                    )
                    nc.vector.reciprocal(rsum[:, mc : mc + 1], sums[:, mc : mc + 1])

                def v_chunk(mc):
                    # v[m, d] for m-chunk mc: lhsT = xT (c,m), rhs = wv (c,d);
                    # +bv via DVE broadcast-add in PSUM, then one ACT copy
                    # applies the 1/colsum fold (per-partition scale).
                    pv = psV.tile([P, 1024], F32, tag="pv")
                    for dh in range(2):
                        for cc in range(CCH):
                            nc.tensor.matmul(
                                pv[:, dh * 512 : (dh + 1) * 512],
                                xT[:, cc, mc * P : (mc + 1) * P],
                                wv_sb[:, cc, dh * 512 : (dh + 1) * 512],
                                start=(cc == 0),
                                stop=(cc == CCH - 1),
                            )
                    nc.vector.tensor_tensor(
                        pv[:], pv[:], bv_rep[:], mybir.AluOpType.add
                    )
                    nc.scalar.activation(
                        out=v[:, mc, :],
                        in_=pv[:],
                        func=Copy,
                        scale=rsum[:, mc : mc + 1],
                    )

                with nc.named_scope("C_loop"):
                    for mc in range(MT):
                        dots_chunk(mc)
                        v_chunk(mc)
                psC_stack.close()

                # ---------- Phase D: out = attnT^T @ v' ----------
                with (
                    tc.tile_pool(name="ps_d", bufs=4, space="PSUM") as psD,
                    tc.tile_pool(name="so", bufs=4) as so,
                    nc.named_scope("D_out"),
                ):
                    for ntc in range(NT):
                        for dh in range(2):
                            po = psD.tile([P, 512], F32, tag="po")
                            for mc in range(MT):
                                nc.tensor.matmul(
                                    po[:],
                                    attnT[:, mc, ntc * P : (ntc + 1) * P],
                                    v[:, mc, dh * 512 : (dh + 1) * 512],
                                    start=(mc == 0),
                                    stop=(mc == MT - 1),
                                )
                            ot = so.tile([P, 512], F32, tag="ot")
                            nc.scalar.copy(ot[:], po[:])
                            eng = nc.sync if (ntc + dh) % 2 == 0 else nc.gpsimd
                            eng.dma_start(
                                out_r[:, ntc, dh * 512 : (dh + 1) * 512], ot[:]
                            )

    nc.finalize()
    return nc


def _get_nc():
    if "nc" not in _CACHE:
        _CACHE["nc"] = _build_nc()
    return _CACHE["nc"]


def _prep_in_maps(y, x, Wq, bq, Wk, bk, Wv, bv):
    y = np.asarray(y, dtype=np.float32)
    x = np.asarray(x, dtype=np.float32)
    # pre-transpose + fp16-cast activations on host (same rounding the
    # previous in-flight DMA cast applied, just done before the transpose)
    yts = [np.ascontiguousarray(y[b].T.astype(np.float16)) for b in range(B)]
    xts = [np.ascontiguousarray(x[b].T.astype(np.float16)) for b in range(B)]
    wqt = np.ascontiguousarray((np.asarray(Wq) * SCALE).T.astype(np.float16))
    wkt = np.ascontiguousarray(np.asarray(Wk).T.astype(np.float16))
    wvt = np.ascontiguousarray(np.asarray(Wv).T.astype(np.float16))
    bq32 = (np.asarray(bq) * SCALE).astype(np.float32)
    bk32 = np.asarray(bk, dtype=np.float32)
    bv32 = np.asarray(bv, dtype=np.float32)
    return [
        {
            "yt": yts[b],
            "xt": xts[b],
            "wqt": wqt,
            "wkt": wkt,
            "wvt": wvt,
            "bq": bq32,
            "bk": bk32,
            "bv": bv32,
        }
        for b in range(B)
    ]


def run(inputs, trace=False, trace_cores=None):
    nc = _get_nc()
    in_maps = _prep_in_maps(**inputs)
    r = run_bass_kernel_spmd(
        nc, in_maps, list(range(B)), trace=trace, trace_cores=trace_cores
    )
    out = np.stack([r.results[b]["out"] for b in range(B)], axis=0)
    return out, r


def kernel(**inputs) -> np.ndarray:
    out, _ = run(inputs, trace=False)
    return out


# revision 9
# speedup vs baseline: 1.0246x; 1.0246x over previous
"""Cross-attention (softmax over queries) on 8 Trainium2 NeuronCores.

Reference (per batch b):
    q = y @ Wq.T + bq            [N, H]
    k = x @ Wk.T + bk            [M, H]
    v = x @ Wv.T + bv            [M, D]
    dots = (q @ k.T) * H**-0.5   [N, M]
    attn = softmax(dots, axis=0) (over queries n, per key column m)
    out  = attn @ v              [N, D]

Sharding: data-parallel over batch B=8, one batch per core (SPMD).

Host prep: y/x are shipped pre-transposed ([C, N] / [C, M]) and pre-cast to
fp16 (the same rounding the previous in-flight DMA cast applied); weights are
pre-transposed/pre-scaled fp16. All matmuls run fp16 with fp32 PSUM.

Device algorithm (per core):
  A. DMA yT/xT quarters; project qT[h,n], kT[h,m] per quarter (q/k biases
     added by the ACT psum->sbuf copy, per-partition).
  C. per 128-row key chunk mc: dotsT[m,n] into two [128,1024] PSUM halves,
     column max (DVE), fused exp+rowsum on ACT into attnT fp16; then
     V-projection chunk, +bv via DVE broadcast-add in PSUM, single ACT
     copy applies the 1/colsum fold (scale=rsum per-partition) -> v fp16.
  D. out[n,d] = sum_m attnT[m,n] * v'[m,d]; dense 16-matmul PSUM chains.
"""

from contextlib import ExitStack

import numpy as np

import concourse.mybir as mybir
import concourse.tile as tile
from concourse import bacc
from concourse.bass_utils import run_bass_kernel_spmd

F32 = mybir.dt.float32
F16 = mybir.dt.float16
Exp = mybir.ActivationFunctionType.Exp
Copy = mybir.ActivationFunctionType.Copy
AX = mybir.AxisListType.X

B, N, M, C, H, D = 8, 2048, 2048, 1024, 512, 1024
P = 128
NT, MT, CCH, HC = N // P, M // P, C // P, H // P  # 16, 16, 8, 4
SCALE = (C // 2) ** -0.5

_CACHE = {}


def _build_nc():
    nc = bacc.Bacc("TRN2", target_bir_lowering=False, debug=False)

    yt_d = nc.dram_tensor("yt", [C, N], F16, kind="ExternalInput").ap()
    xt_d = nc.dram_tensor("xt", [C, M], F16, kind="ExternalInput").ap()
    wqt_d = nc.dram_tensor("wqt", [C, H], F16, kind="ExternalInput").ap()
    wkt_d = nc.dram_tensor("wkt", [C, H], F16, kind="ExternalInput").ap()
    wvt_d = nc.dram_tensor("wvt", [C, D], F16, kind="ExternalInput").ap()
    bq_d = nc.dram_tensor("bq", [H], F32, kind="ExternalInput").ap()
    bk_d = nc.dram_tensor("bk", [H], F32, kind="ExternalInput").ap()
    bv_d = nc.dram_tensor("bv", [D], F32, kind="ExternalInput").ap()
    out_d = nc.dram_tensor("out", [N, D], F32, kind="ExternalOutput").ap()

    yt_r = yt_d.rearrange("(o p) n -> p o n", p=P)  # [128, 8, 2048]
    xt_r = xt_d.rearrange("(o p) n -> p o n", p=P)
    out_r = out_d.rearrange("(t p) d -> p t d", p=P)

    with tile.TileContext(nc) as tc:
        with (
            tc.tile_pool(name="persist", bufs=1) as pers,
            tc.tile_pool(name="stats", bufs=1) as stats,
            tc.tile_pool(name="xT_pool", bufs=1) as xTp,
        ):
            qT = pers.tile([P, HC, N], F16, tag="qT")  # [h%128, h//128, n] 2MB
            kT = pers.tile([P, HC, M], F16, tag="kT")  # 2MB
            v = pers.tile([P, MT, D], F16, tag="v")  # [m%128, m//128, d] 4MB
            wv_sb = pers.tile([P, CCH, D], F16, tag="wv")  # 2MB
            bv_rep = pers.tile([P, D], F32, tag="bv_rep")  # 4KB/part

            sums = stats.tile([P, MT], F32, tag="sums")
            rsum = stats.tile([P, MT], F32, tag="rsum")
            bq_sb = stats.tile([P, HC], F32, tag="bq")  # [h%128, h//128]
            bk_sb = stats.tile([P, HC], F32, tag="bk")
            bv_sb = stats.tile([1, D], F32, tag="bv")
            warm = stats.tile([P, 512], F16, tag="warm")

            xT = xTp.tile([P, CCH, M], F16, tag="xT")  # alive through phase C

            # ---------- Phase A: load yT/xT + q/k projections ----------
            ps_stack = ExitStack()
            psPP = ps_stack.enter_context(
                tc.tile_pool(name="ps_pp", bufs=4, space="PSUM")
            )
            with (
                tc.tile_pool(name="yT_pool", bufs=1) as yTp,
                tc.tile_pool(name="w_pool", bufs=1) as wp,
                tc.tile_pool(name="ps_w", bufs=1, space="PSUM") as psW,
            ):
                wq_sb = wp.tile([P, CCH, H], F16, tag="wq")  # [c%128, c//128, h]
                wk_sb = wp.tile([P, CCH, H], F16, tag="wk")
                nc.sync.dma_start(wq_sb[:], wqt_d.rearrange("(o p) h -> p o h", p=P))
                nc.sync.dma_start(wk_sb[:], wkt_d.rearrange("(o p) h -> p o h", p=P))
                nc.sync.dma_start(bq_sb[:], bq_d.rearrange("(o p) -> p o", p=P))
                nc.sync.dma_start(bk_sb[:], bk_d.rearrange("(o p) -> p o", p=P))
                nc.sync.dma_start(bv_sb[:], bv_d[None, :])

                yT = yTp.tile([P, CCH, N], F16, tag="yT")
                # issue all activation loads up front; they pipeline on the
                # gpsimd queue while projections consume earlier quarters
                for j in range(4):
                    nc.gpsimd.dma_start(
                        yT[:, :, j * 512 : (j + 1) * 512],
                        yt_r[:, :, j * 512 : (j + 1) * 512],
                    )
                for j in range(4):
                    nc.gpsimd.dma_start(
                        xT[:, :, j * 512 : (j + 1) * 512],
                        xt_r[:, :, j * 512 : (j + 1) * 512],
                    )
                nc.gpsimd.partition_broadcast(bv_rep[:], bv_sb[:1, :])
                nc.sync.dma_start(wv_sb[:], wvt_d.rearrange("(o p) d -> p o d", p=P))

                # PE warm-up: ramp the clock on junk matmuls while DMAs land
                nc.vector.memset(warm[:], 0.0)
                pw = psW.tile([P, 512], F32, tag="pw")
                with nc.named_scope("A_warm"):
                    for i in range(20):
                        nc.tensor.matmul(
                            pw[:], warm[:, :P], warm[:], start=(i == 0),
                            stop=(i == 19),
                        )

                def project_j(dst, w_sb, b_sb, src_T, j):
                    # one 512-wide column block of a projection, all hc chunks
                    for hc in range(HC):
                        pp = psPP.tile([P, 512], F32, tag="pp")
                        for cc in range(CCH):
                            nc.tensor.matmul(
                                pp[:],
                                w_sb[:, cc, hc * P : (hc + 1) * P],
                                src_T[:, cc, j * 512 : (j + 1) * 512],
                                start=(cc == 0),
                                stop=(cc == CCH - 1),
                            )
                        # ACT copy: psum -> f16, + per-partition bias
                        nc.scalar.add(
                            dst[:, hc, j * 512 : (j + 1) * 512],
                            pp[:],
                            b_sb[:, hc : hc + 1],
                        )

                with nc.named_scope("A_yq"):
                    for j in range(4):
                        project_j(qT, wq_sb, bq_sb, yT, j)
                with nc.named_scope("A_xk"):
                    for j in range(4):
                        project_j(kT, wk_sb, bk_sb, xT, j)

            ps_stack.close()  # free phase-A psum pool before phase C

            # ---------- Phase C: dots/softmax then V-proj chunks ----------
            with (
                tc.tile_pool(name="late", bufs=1) as late,
                tc.tile_pool(name="sc", bufs=4) as sc,
            ):
                psC_stack = ExitStack()
                psC = psC_stack.enter_context(
                    tc.tile_pool(name="ps_c", bufs=1, space="PSUM")
                )
                psV = psC_stack.enter_context(
                    tc.tile_pool(name="ps_v", bufs=2, space="PSUM")
                )
                attnT = late.tile([P, MT, N], F16, tag="attnT")  # 8MB

                def dots_chunk(mc):
                    halves = []
                    for h in range(2):
                        pd = psC.tile([P, 1024], F32, tag=f"dots{h}")
                        for j2 in range(2):
                            j = h * 2 + j2
                            for hc in range(HC):
                                nc.tensor.matmul(
                                    pd[:, j2 * 512 : (j2 + 1) * 512],
                                    kT[:, hc, mc * P : (mc + 1) * P],
                                    qT[:, hc, j * 512 : (j + 1) * 512],
                                    start=(hc == 0),
                                    stop=(hc == HC - 1),
                                )
                        halves.append(pd)
                    pmax = sc.tile([P, 2], F32, tag="pmax")
                    for h in range(2):
                        nc.vector.reduce_max(
                            pmax[:, h : h + 1], halves[h][:], axis=AX
                        )
                    negmax = sc.tile([P, 1], F32, tag="negmax")
                    nc.vector.reduce_max(negmax[:], pmax[:], axis=AX, negate=True)
                    ssum = sc.tile([P, 2], F32, tag="ssum")
                    for h in range(2):
                        nc.scalar.activation(
                            out=attnT[:, mc, h * 1024 : (h + 1) * 1024],
                            in_=halves[h][:],
                            func=Exp,
                            bias=negmax[:],
                            accum_out=ssum[:, h : h + 1],
                        )
                    nc.vector.tensor_tensor(
                        sums[:, mc : mc + 1],
                        ssum[:, 0:1],
                        ssum[:, 1:2],
                        mybir.AluOpType.add,
                    )
                    nc.vector.reciprocal(rsum[:, mc : mc + 1], sums[:, mc : mc + 1])

                def v_chunk(mc):
                    # v[m, d] for m-chunk mc: lhsT = xT (c,m), rhs = wv (c,d);
                    # +bv via DVE broadcast-add in PSUM, then one ACT copy
                    # applies the 1/colsum fold (per-partition scale).
                    pv = psV.tile([P, 1024], F32, tag="pv")
                    for dh in range(2):
                        for cc in range(CCH):
                            nc.tensor.matmul(
                                pv[:, dh * 512 : (dh + 1) * 512],
                                xT[:, cc, mc * P : (mc + 1) * P],
                                wv_sb[:, cc, dh * 512 : (dh + 1) * 512],
                                start=(cc == 0),
                                stop=(cc == CCH - 1),
                            )
                    nc.vector.tensor_tensor(
                        pv[:], pv[:], bv_rep[:], mybir.AluOpType.add
                    )
                    nc.scalar.activation(
                        out=v[:, mc, :],
                        in_=pv[:],
                        func=Copy,
                        scale=rsum[:, mc : mc + 1],
                    )

                with nc.named_scope("C_loop"):
                    for mc in range(MT):
                        dots_chunk(mc)
                        v_chunk(mc)
                psC_stack.close()

                # ---------- Phase D: out = attnT^T @ v' ----------
                with (
                    tc.tile_pool(name="ps_d", bufs=4, space="PSUM") as psD,
                    tc.tile_pool(name="so", bufs=4) as so,
                    nc.named_scope("D_out"),
                ):
                    for ntc in range(NT):
                        ot = so.tile([P, D], F32, tag="ot")
                        for dh in range(2):
                            po = psD.tile([P, 512], F32, tag="po")
                            for mc in range(MT):
                                nc.tensor.matmul(
                                    po[:],
                                    attnT[:, mc, ntc * P : (ntc + 1) * P],
                                    v[:, mc, dh * 512 : (dh + 1) * 512],
                                    start=(mc == 0),
                                    stop=(mc == MT - 1),
                                )
                            nc.scalar.copy(
                                ot[:, dh * 512 : (dh + 1) * 512], po[:]
                            )
                        eng = nc.sync if ntc % 2 == 0 else nc.gpsimd
                        eng.dma_start(out_r[:, ntc, :], ot[:])

    nc.finalize()
    return nc


def _get_nc():
    if "nc" not in _CACHE:
        _CACHE["nc"] = _build_nc()
    return _CACHE["nc"]


def _prep_in_maps(y, x, Wq, bq, Wk, bk, Wv, bv):
    y = np.asarray(y, dtype=np.float32)
    x = np.asarray(x, dtype=np.float32)
    # pre-transpose + fp16-cast activations on host (same rounding the
    # previous in-flight DMA cast applied, just done before the transpose)
    yts = [np.ascontiguousarray(y[b].T.astype(np.float16)) for b in range(B)]
    xts = [np.ascontiguousarray(x[b].T.astype(np.float16)) for b in range(B)]
    wqt = np.ascontiguousarray((np.asarray(Wq) * SCALE).T.astype(np.float16))
    wkt = np.ascontiguousarray(np.asarray(Wk).T.astype(np.float16))
    wvt = np.ascontiguousarray(np.asarray(Wv).T.astype(np.float16))
    bq32 = (np.asarray(bq) * SCALE).astype(np.float32)
    bk32 = np.asarray(bk, dtype=np.float32)
    bv32 = np.asarray(bv, dtype=np.float32)
    return [
        {
            "yt": yts[b],
            "xt": xts[b],
            "wqt": wqt,
            "wkt": wkt,
            "wvt": wvt,
            "bq": bq32,
            "bk": bk32,
            "bv": bv32,
        }
        for b in range(B)
    ]


def run(inputs, trace=False, trace_cores=None):
    nc = _get_nc()
    in_maps = _prep_in_maps(**inputs)
    r = run_bass_kernel_spmd(
        nc, in_maps, list(range(B)), trace=trace, trace_cores=trace_cores
    )
    out = np.stack([r.results[b]["out"] for b in range(B)], axis=0)
    return out, r


def kernel(**inputs) -> np.ndarray:
    out, _ = run(inputs, trace=False)
    return out


# revision 11
# speedup vs baseline: 1.0708x; 1.0451x over previous
"""Cross-attention (softmax over queries) on 8 Trainium2 NeuronCores.

Reference (per batch b):
    q = y @ Wq.T + bq            [N, H]
    k = x @ Wk.T + bk            [M, H]
    v = x @ Wv.T + bv            [M, D]
    dots = (q @ k.T) * H**-0.5   [N, M]
    attn = softmax(dots, axis=0) (over queries n, per key column m)
    out  = attn @ v              [N, D]

Sharding: data-parallel over batch B=8, one batch per core (SPMD).

Host prep: y/x are shipped pre-transposed ([C, N] / [C, M]) and pre-cast to
fp16 (the same rounding the previous in-flight DMA cast applied); weights are
pre-transposed/pre-scaled fp16. All matmuls run fp16 with fp32 PSUM.

Device algorithm (per core):
  A. DMA yT/xT quarters; project qT[h,n], kT[h,m] per quarter (q/k biases
     added by the ACT psum->sbuf copy, per-partition).
  C. per 128-row key chunk mc: dotsT[m,n] into two [128,1024] PSUM halves,
     column max (DVE), fused exp+rowsum on ACT into attnT fp16; then
     V-projection chunk, +bv via DVE broadcast-add in PSUM, single ACT
     copy applies the 1/colsum fold (scale=rsum per-partition) -> v fp16.
  D. out[n,d] = sum_m attnT[m,n] * v'[m,d]; dense 16-matmul PSUM chains.
"""

from contextlib import ExitStack

import numpy as np

import concourse.mybir as mybir
import concourse.tile as tile
from concourse import bacc
from concourse.bass_utils import run_bass_kernel_spmd

F32 = mybir.dt.float32
F16 = mybir.dt.float16
Exp = mybir.ActivationFunctionType.Exp
Copy = mybir.ActivationFunctionType.Copy
AX = mybir.AxisListType.X

B, N, M, C, H, D = 8, 2048, 2048, 1024, 512, 1024
P = 128
NT, MT, CCH, HC = N // P, M // P, C // P, H // P  # 16, 16, 8, 4
SCALE = (C // 2) ** -0.5

_CACHE = {}


def _build_nc():
    nc = bacc.Bacc("TRN2", target_bir_lowering=False, debug=False)

    yt_d = nc.dram_tensor("yt", [C, N], F16, kind="ExternalInput").ap()
    xt_d = nc.dram_tensor("xt", [C, M], F16, kind="ExternalInput").ap()
    wqt_d = nc.dram_tensor("wqt", [C, H], F16, kind="ExternalInput").ap()
    wkt_d = nc.dram_tensor("wkt", [C, H], F16, kind="ExternalInput").ap()
    wvt_d = nc.dram_tensor("wvt", [C, D], F16, kind="ExternalInput").ap()
    bq_d = nc.dram_tensor("bq", [H], F32, kind="ExternalInput").ap()
    bk_d = nc.dram_tensor("bk", [H], F32, kind="ExternalInput").ap()
    bv_d = nc.dram_tensor("bv", [D], F32, kind="ExternalInput").ap()
    out_d = nc.dram_tensor("out", [N, D], F32, kind="ExternalOutput").ap()

    yt_r = yt_d.rearrange("(o p) n -> p o n", p=P)  # [128, 8, 2048]
    xt_r = xt_d.rearrange("(o p) n -> p o n", p=P)
    out_r = out_d.rearrange("(t p) d -> p t d", p=P)

    with tile.TileContext(nc) as tc:
        with (
            tc.tile_pool(name="persist", bufs=1) as pers,
            tc.tile_pool(name="stats", bufs=1) as stats,
            tc.tile_pool(name="xT_pool", bufs=1) as xTp,
        ):
            qT = pers.tile([P, HC, N], F16, tag="qT")  # [h%128, h//128, n] 2MB
            kT = pers.tile([P, HC, M], F16, tag="kT")  # 2MB
            v = pers.tile([P, MT, D], F16, tag="v")  # [m%128, m//128, d] 4MB
            wv_sb = pers.tile([P, CCH, D], F16, tag="wv")  # 2MB
            bv_rep = pers.tile([P, D], F32, tag="bv_rep")  # 4KB/part

            sums = stats.tile([P, MT], F32, tag="sums")
            rsum = stats.tile([P, MT], F32, tag="rsum")
            bq_sb = stats.tile([P, HC], F32, tag="bq")  # [h%128, h//128]
            bk_sb = stats.tile([P, HC], F32, tag="bk")
            bv_sb = stats.tile([1, D], F32, tag="bv")
            nc.sync.dma_start(bq_sb[:], bq_d.rearrange("(o p) -> p o", p=P))
            nc.sync.dma_start(bk_sb[:], bk_d.rearrange("(o p) -> p o", p=P))
            nc.sync.dma_start(bv_sb[:], bv_d[None, :])
            nc.gpsimd.partition_broadcast(bv_rep[:], bv_sb[:1, :])

            xT = xTp.tile([P, CCH, M], F16, tag="xT")  # alive through phase C

            # ---------- Phase A: load yT/xT + q/k projections ----------
            ps_stack = ExitStack()
            psPP = ps_stack.enter_context(
                tc.tile_pool(name="ps_pp", bufs=4, space="PSUM")
            )
            with (
                tc.tile_pool(name="yT_pool", bufs=1) as yTp,
                tc.tile_pool(name="w_pool", bufs=1) as wp,
            ):
                wq_sb = wp.tile([P, CCH, H], F16, tag="wq")  # [c%128, c//128, h]
                wk_sb = wp.tile([P, CCH, H], F16, tag="wk")
                nc.sync.dma_start(wq_sb[:], wqt_d.rearrange("(o p) h -> p o h", p=P))
                nc.sync.dma_start(wk_sb[:], wkt_d.rearrange("(o p) h -> p o h", p=P))

                yT = yTp.tile([P, CCH, N], F16, tag="yT")
                # issue all activation loads up front; they pipeline on the
                # gpsimd queue while projections consume earlier quarters
                for j in range(4):
                    nc.gpsimd.dma_start(
                        yT[:, :, j * 512 : (j + 1) * 512],
                        yt_r[:, :, j * 512 : (j + 1) * 512],
                    )
                for j in range(4):
                    nc.gpsimd.dma_start(
                        xT[:, :, j * 512 : (j + 1) * 512],
                        xt_r[:, :, j * 512 : (j + 1) * 512],
                    )
                nc.sync.dma_start(wv_sb[:], wvt_d.rearrange("(o p) d -> p o d", p=P))

                def project_j(dst, w_sb, b_sb, src_T, j):
                    # one 512-wide column block of a projection, all hc chunks
                    for hc in range(HC):
                        pp = psPP.tile([P, 512], F32, tag="pp")
                        for cc in range(CCH):
                            nc.tensor.matmul(
                                pp[:],
                                w_sb[:, cc, hc * P : (hc + 1) * P],
                                src_T[:, cc, j * 512 : (j + 1) * 512],
                                start=(cc == 0),
                                stop=(cc == CCH - 1),
                            )
                        # ACT copy: psum -> f16, + per-partition bias
                        nc.scalar.add(
                            dst[:, hc, j * 512 : (j + 1) * 512],
                            pp[:],
                            b_sb[:, hc : hc + 1],
                        )

                with nc.named_scope("A_yq"):
                    for j in range(4):
                        project_j(qT, wq_sb, bq_sb, yT, j)
                with nc.named_scope("A_xk"):
                    for j in range(4):
                        project_j(kT, wk_sb, bk_sb, xT, j)

            ps_stack.close()  # free phase-A psum pool before phase C

            # ---------- Phase C: dots/softmax then V-proj chunks ----------
            with (
                tc.tile_pool(name="late", bufs=1) as late,
                tc.tile_pool(name="sc", bufs=4) as sc,
            ):
                psC_stack = ExitStack()
                psC = psC_stack.enter_context(
                    tc.tile_pool(name="ps_c", bufs=1, space="PSUM")
                )
                psV = psC_stack.enter_context(
                    tc.tile_pool(name="ps_v", bufs=2, space="PSUM")
                )
                attnT = late.tile([P, MT, N], F16, tag="attnT")  # 8MB

                def dots_chunk(mc):
                    halves = []
                    for h in range(2):
                        pd = psC.tile([P, 1024], F32, tag=f"dots{h}")
                        for j2 in range(2):
                            j = h * 2 + j2
                            for hc in range(HC):
                                nc.tensor.matmul(
                                    pd[:, j2 * 512 : (j2 + 1) * 512],
                                    kT[:, hc, mc * P : (mc + 1) * P],
                                    qT[:, hc, j * 512 : (j + 1) * 512],
                                    start=(hc == 0),
                                    stop=(hc == HC - 1),
                                )
                        halves.append(pd)
                    pmax = sc.tile([P, 2], F32, tag="pmax")
                    for h in range(2):
                        nc.vector.reduce_max(
                            pmax[:, h : h + 1], halves[h][:], axis=AX
                        )
                    negmax = sc.tile([P, 1], F32, tag="negmax")
                    nc.vector.reduce_max(negmax[:], pmax[:], axis=AX, negate=True)
                    ssum = sc.tile([P, 2], F32, tag="ssum")
                    for h in range(2):
                        nc.scalar.activation(
                            out=attnT[:, mc, h * 1024 : (h + 1) * 1024],
                            in_=halves[h][:],
                            func=Exp,
                            bias=negmax[:],
                            accum_out=ssum[:, h : h + 1],
                        )
                    nc.vector.tensor_tensor(
                        sums[:, mc : mc + 1],
                        ssum[:, 0:1],
                        ssum[:, 1:2],
                        mybir.AluOpType.add,
                    )
                    nc.vector.reciprocal(rsum[:, mc : mc + 1], sums[:, mc : mc + 1])

                def v_chunk(mc):
                    # v[m, d] for m-chunk mc: lhsT = xT (c,m), rhs = wv (c,d);
                    # +bv via DVE broadcast-add in PSUM, then one ACT copy
                    # applies the 1/colsum fold (per-partition scale).
                    pv = psV.tile([P, 1024], F32, tag="pv")
                    for dh in range(2):
                        for cc in range(CCH):
                            nc.tensor.matmul(
                                pv[:, dh * 512 : (dh + 1) * 512],
                                xT[:, cc, mc * P : (mc + 1) * P],
                                wv_sb[:, cc, dh * 512 : (dh + 1) * 512],
                                start=(cc == 0),
                                stop=(cc == CCH - 1),
                            )
                    nc.vector.tensor_tensor(
                        pv[:], pv[:], bv_rep[:], mybir.AluOpType.add
                    )
                    nc.scalar.activation(
                        out=v[:, mc, :],
                        in_=pv[:],
                        func=Copy,
                        scale=rsum[:, mc : mc + 1],
                    )

                with nc.named_scope("C_loop"):
                    for mc in range(MT):
                        dots_chunk(mc)
                        v_chunk(mc)
                psC_stack.close()

                # ---------- Phase D: out = attnT^T @ v' ----------
                with (
                    tc.tile_pool(name="ps_d", bufs=4, space="PSUM") as psD,
                    tc.tile_pool(name="so", bufs=4) as so,
                    nc.named_scope("D_out"),
                ):
                    for ntc in range(NT):
                        for dh in range(2):
                            po = psD.tile([P, 512], F32, tag="po")
                            for mc in range(MT):
                                nc.tensor.matmul(
                                    po[:],
                                    attnT[:, mc, ntc * P : (ntc + 1) * P],
                                    v[:, mc, dh * 512 : (dh + 1) * 512],
                                    start=(mc == 0),
                                    stop=(mc == MT - 1),
                                )
                            ot = so.tile([P, 512], F32, tag="ot")
                            nc.scalar.copy(ot[:], po[:])
                            nc.sync.dma_start(
                                out_r[:, ntc, dh * 512 : (dh + 1) * 512], ot[:]
                            )

    nc.finalize()
    return nc


def _get_nc():
    if "nc" not in _CACHE:
        _CACHE["nc"] = _build_nc()
    return _CACHE["nc"]


def _prep_in_maps(y, x, Wq, bq, Wk, bk, Wv, bv):
    y = np.asarray(y, dtype=np.float32)
    x = np.asarray(x, dtype=np.float32)
    # pre-transpose + fp16-cast activations on host (same rounding the
    # previous in-flight DMA cast applied, just done before the transpose)
    yts = [np.ascontiguousarray(y[b].T.astype(np.float16)) for b in range(B)]
    xts = [np.ascontiguousarray(x[b].T.astype(np.float16)) for b in range(B)]
    wqt = np.ascontiguousarray((np.asarray(Wq) * SCALE).T.astype(np.float16))
    wkt = np.ascontiguousarray(np.asarray(Wk).T.astype(np.float16))
    wvt = np.ascontiguousarray(np.asarray(Wv).T.astype(np.float16))
    bq32 = (np.asarray(bq) * SCALE).astype(np.float32)
    bk32 = np.asarray(bk, dtype=np.float32)
    bv32 = np.asarray(bv, dtype=np.float32)
    return [
        {
            "yt": yts[b],
            "xt": xts[b],
            "wqt": wqt,
            "wkt": wkt,
            "wvt": wvt,
            "bq": bq32,
            "bk": bk32,
            "bv": bv32,
        }
        for b in range(B)
    ]


def run(inputs, trace=False, trace_cores=None):
    nc = _get_nc()
    in_maps = _prep_in_maps(**inputs)
    r = run_bass_kernel_spmd(
        nc, in_maps, list(range(B)), trace=trace, trace_cores=trace_cores
    )
    out = np.stack([r.results[b]["out"] for b in range(B)], axis=0)
    return out, r


def kernel(**inputs) -> np.ndarray:
    out, _ = run(inputs, trace=False)
    return out


# revision 13
# speedup vs baseline: 1.0826x; 1.0110x over previous
"""Cross-attention (softmax over queries) on 8 Trainium2 NeuronCores.

Reference (per batch b):
    q = y @ Wq.T + bq            [N, H]
    k = x @ Wk.T + bk            [M, H]
    v = x @ Wv.T + bv            [M, D]
    dots = (q @ k.T) * H**-0.5   [N, M]
    attn = softmax(dots, axis=0) (over queries n, per key column m)
    out  = attn @ v              [N, D]

Sharding: data-parallel over batch B=8, one batch per core (SPMD).

Host prep: y/x are shipped pre-transposed ([C, N] / [C, M]) and pre-cast to
fp16 (the same rounding the previous in-flight DMA cast applied); weights are
pre-transposed/pre-scaled fp16. All matmuls run fp16 with fp32 PSUM.

Device algorithm (per core):
  A. DMA yT/xT quarters; project qT[h,n], kT[h,m] per quarter (q/k biases
     added by the ACT psum->sbuf copy, per-partition).
  C. per 128-row key chunk mc: dotsT[m,n] into two [128,1024] PSUM halves,
     column max (DVE), fused exp+rowsum on ACT into attnT fp16; then
     V-projection chunk, +bv via DVE broadcast-add in PSUM, single ACT
     copy applies the 1/colsum fold (scale=rsum per-partition) -> v fp16.
  D. out[n,d] = sum_m attnT[m,n] * v'[m,d]; dense 16-matmul PSUM chains.
"""

from contextlib import ExitStack

import numpy as np

import concourse.mybir as mybir
import concourse.tile as tile
from concourse import bacc
from concourse.bass_utils import run_bass_kernel_spmd

F32 = mybir.dt.float32
F16 = mybir.dt.float16
Exp = mybir.ActivationFunctionType.Exp
Copy = mybir.ActivationFunctionType.Copy
AX = mybir.AxisListType.X

B, N, M, C, H, D = 8, 2048, 2048, 1024, 512, 1024
P = 128
NT, MT, CCH, HC = N // P, M // P, C // P, H // P  # 16, 16, 8, 4
SCALE = (C // 2) ** -0.5

_CACHE = {}


def _build_nc():
    nc = bacc.Bacc("TRN2", target_bir_lowering=False, debug=False)

    yt_d = nc.dram_tensor("yt", [C, N], F16, kind="ExternalInput").ap()
    xt_d = nc.dram_tensor("xt", [C, M], F16, kind="ExternalInput").ap()
    wqt_d = nc.dram_tensor("wqt", [C, H], F16, kind="ExternalInput").ap()
    wkt_d = nc.dram_tensor("wkt", [C, H], F16, kind="ExternalInput").ap()
    wvt_d = nc.dram_tensor("wvt", [C, D], F16, kind="ExternalInput").ap()
    bq_d = nc.dram_tensor("bq", [H], F32, kind="ExternalInput").ap()
    bk_d = nc.dram_tensor("bk", [H], F32, kind="ExternalInput").ap()
    bv_d = nc.dram_tensor("bv", [D], F32, kind="ExternalInput").ap()
    out_d = nc.dram_tensor("out", [N, D], F32, kind="ExternalOutput").ap()

    yt_r = yt_d.rearrange("(o p) n -> p o n", p=P)  # [128, 8, 2048]
    xt_r = xt_d.rearrange("(o p) n -> p o n", p=P)
    out_r = out_d.rearrange("(t p) d -> p t d", p=P)

    with tile.TileContext(nc) as tc:
        with (
            tc.tile_pool(name="persist", bufs=1) as pers,
            tc.tile_pool(name="stats", bufs=1) as stats,
            tc.tile_pool(name="xT_pool", bufs=1) as xTp,
        ):
            qT = pers.tile([P, HC, N], F16, tag="qT")  # [h%128, h//128, n] 2MB
            kT = pers.tile([P, HC, M], F16, tag="kT")  # 2MB
            v = pers.tile([P, MT, D], F16, tag="v")  # [m%128, m//128, d] 4MB
            wv_sb = pers.tile([P, CCH, D], F16, tag="wv")  # 2MB
            bv_rep = pers.tile([P, D], F32, tag="bv_rep")  # 4KB/part

            sums = stats.tile([P, MT], F32, tag="sums")
            rsum = stats.tile([P, MT], F32, tag="rsum")
            bq_sb = stats.tile([P, HC], F32, tag="bq")  # [h%128, h//128]
            bk_sb = stats.tile([P, HC], F32, tag="bk")
            bv_sb = stats.tile([1, D], F32, tag="bv")
            warm = stats.tile([P, 512], F16, tag="warm")

            xT = xTp.tile([P, CCH, M], F16, tag="xT")  # alive through phase C

            # ---------- Phase A: load yT/xT + q/k projections ----------
            ps_stack = ExitStack()
            psPP = ps_stack.enter_context(
                tc.tile_pool(name="ps_pp", bufs=4, space="PSUM")
            )
            with (
                tc.tile_pool(name="yT_pool", bufs=1) as yTp,
                tc.tile_pool(name="w_pool", bufs=1) as wp,
                tc.tile_pool(name="ps_w", bufs=1, space="PSUM") as psW,
            ):
                wq_sb = wp.tile([P, CCH, H], F16, tag="wq")  # [c%128, c//128, h]
                wk_sb = wp.tile([P, CCH, H], F16, tag="wk")
                yT = yTp.tile([P, CCH, N], F16, tag="yT")
                wq_r = wqt_d.rearrange("(o p) h -> p o h", p=P)
                # fan the critical first loads (wq, y quarter 0) across the
                # three DMA-capable queues (sync/SP, scalar/ACT, gpsimd) so
                # the first projection chain starts ~13us in instead of ~22us
                nc.sync.dma_start(wq_sb[:, 0:4, :], wq_r[:, 0:4, :])
                nc.scalar.dma_start(wq_sb[:, 4:8, :], wq_r[:, 4:8, :])
                nc.gpsimd.dma_start(yT[:, 0:4, 0:512], yt_r[:, 0:4, 0:512])
                nc.gpsimd.dma_start(yT[:, 4:8, 0:512], yt_r[:, 4:8, 0:512])
                for j in range(1, 4):
                    nc.gpsimd.dma_start(
                        yT[:, :, j * 512 : (j + 1) * 512],
                        yt_r[:, :, j * 512 : (j + 1) * 512],
                    )
                nc.scalar.dma_start(bq_sb[:], bq_d.rearrange("(o p) -> p o", p=P))
                nc.scalar.dma_start(bk_sb[:], bk_d.rearrange("(o p) -> p o", p=P))
                nc.scalar.dma_start(bv_sb[:], bv_d[None, :])
                nc.sync.dma_start(wk_sb[:], wkt_d.rearrange("(o p) h -> p o h", p=P))
                for j in range(4):
                    nc.sync.dma_start(
                        xT[:, :, j * 512 : (j + 1) * 512],
                        xt_r[:, :, j * 512 : (j + 1) * 512],
                    )
                nc.gpsimd.dma_start(wv_sb[:], wvt_d.rearrange("(o p) d -> p o d", p=P))
                nc.gpsimd.partition_broadcast(bv_rep[:], bv_sb[:1, :])

                # PE warm-up on junk data: ramps the clock while DMAs land
                nc.vector.memset(warm[:], 0.0)
                pw = psW.tile([P, 512], F32, tag="pw")
                with nc.named_scope("A_warm"):
                    for i in range(10):
                        nc.tensor.matmul(
                            pw[:], warm[:, :P], warm[:], start=(i == 0),
                            stop=(i == 9),
                        )

                def project_j(dst, w_sb, b_sb, src_T, j):
                    # one 512-wide column block of a projection, all hc chunks
                    for hc in range(HC):
                        pp = psPP.tile([P, 512], F32, tag="pp")
                        for cc in range(CCH):
                            nc.tensor.matmul(
                                pp[:],
                                w_sb[:, cc, hc * P : (hc + 1) * P],
                                src_T[:, cc, j * 512 : (j + 1) * 512],
                                start=(cc == 0),
                                stop=(cc == CCH - 1),
                            )
                        # ACT copy: psum -> f16, + per-partition bias
                        nc.scalar.add(
                            dst[:, hc, j * 512 : (j + 1) * 512],
                            pp[:],
                            b_sb[:, hc : hc + 1],
                        )

                with nc.named_scope("A_yq"):
                    for j in range(4):
                        project_j(qT, wq_sb, bq_sb, yT, j)
                with nc.named_scope("A_xk"):
                    for j in range(4):
                        project_j(kT, wk_sb, bk_sb, xT, j)

            ps_stack.close()  # free phase-A psum pool before phase C

            # ---------- Phase C: dots/softmax then V-proj chunks ----------
            with (
                tc.tile_pool(name="late", bufs=1) as late,
                tc.tile_pool(name="sc", bufs=4) as sc,
            ):
                psC_stack = ExitStack()
                psC = psC_stack.enter_context(
                    tc.tile_pool(name="ps_c", bufs=1, space="PSUM")
                )
                psV = psC_stack.enter_context(
                    tc.tile_pool(name="ps_v", bufs=2, space="PSUM")
                )
                attnT = late.tile([P, MT, N], F16, tag="attnT")  # 8MB

                def dots_chunk(mc):
                    halves = []
                    for h in range(2):
                        pd = psC.tile([P, 1024], F32, tag=f"dots{h}")
                        for j2 in range(2):
                            j = h * 2 + j2
                            for hc in range(HC):
                                nc.tensor.matmul(
                                    pd[:, j2 * 512 : (j2 + 1) * 512],
                                    kT[:, hc, mc * P : (mc + 1) * P],
                                    qT[:, hc, j * 512 : (j + 1) * 512],
                                    start=(hc == 0),
                                    stop=(hc == HC - 1),
                                )
                        halves.append(pd)
                    pmax = sc.tile([P, 2], F32, tag="pmax")
                    for h in range(2):
                        nc.vector.reduce_max(
                            pmax[:, h : h + 1], halves[h][:], axis=AX
                        )
                    negmax = sc.tile([P, 1], F32, tag="negmax")
                    nc.vector.reduce_max(negmax[:], pmax[:], axis=AX, negate=True)
                    ssum = sc.tile([P, 2], F32, tag="ssum")
                    for h in range(2):
                        nc.scalar.activation(
                            out=attnT[:, mc, h * 1024 : (h + 1) * 1024],
                            in_=halves[h][:],
                            func=Exp,
                            bias=negmax[:],
                            accum_out=ssum[:, h : h + 1],
                        )
                    nc.vector.tensor_tensor(
                        sums[:, mc : mc + 1],
                        ssum[:, 0:1],
                        ssum[:, 1:2],
                        mybir.AluOpType.add,
                    )
                    nc.vector.reciprocal(rsum[:, mc : mc + 1], sums[:, mc : mc + 1])

                def v_chunk(mc):
                    # v[m, d] for m-chunk mc: lhsT = xT (c,m), rhs = wv (c,d);
                    # +bv via DVE broadcast-add in PSUM, then one ACT copy
                    # applies the 1/colsum fold (per-partition scale).
                    pv = psV.tile([P, 1024], F32, tag="pv")
                    for dh in range(2):
                        for cc in range(CCH):
                            nc.tensor.matmul(
                                pv[:, dh * 512 : (dh + 1) * 512],
                                xT[:, cc, mc * P : (mc + 1) * P],
                                wv_sb[:, cc, dh * 512 : (dh + 1) * 512],
                                start=(cc == 0),
                                stop=(cc == CCH - 1),
                            )
                    nc.vector.tensor_tensor(
                        pv[:], pv[:], bv_rep[:], mybir.AluOpType.add
                    )
                    nc.scalar.activation(
                        out=v[:, mc, :],
                        in_=pv[:],
                        func=Copy,
                        scale=rsum[:, mc : mc + 1],
                    )

                with nc.named_scope("C_loop"):
                    for mc in range(MT):
                        dots_chunk(mc)
                        v_chunk(mc)
                psC_stack.close()

                # ---------- Phase D: out = attnT^T @ v' ----------
                with (
                    tc.tile_pool(name="ps_d", bufs=4, space="PSUM") as psD,
                    tc.tile_pool(name="so", bufs=4) as so,
                    nc.named_scope("D_out"),
                ):
                    for ntc in range(NT):
                        for dh in range(2):
                            po = psD.tile([P, 512], F32, tag="po")
                            for mc in range(MT):
                                nc.tensor.matmul(
                                    po[:],
                                    attnT[:, mc, ntc * P : (ntc + 1) * P],
                                    v[:, mc, dh * 512 : (dh + 1) * 512],
                                    start=(mc == 0),
                                    stop=(mc == MT - 1),
                                )
                            ot = so.tile([P, 512], F32, tag="ot")
                            nc.scalar.copy(ot[:], po[:])
                            nc.sync.dma_start(
                                out_r[:, ntc, dh * 512 : (dh + 1) * 512], ot[:]
                            )

    nc.finalize()
    return nc


def _get_nc():
    if "nc" not in _CACHE:
        _CACHE["nc"] = _build_nc()
    return _CACHE["nc"]


def _prep_in_maps(y, x, Wq, bq, Wk, bk, Wv, bv):
    y = np.asarray(y, dtype=np.float32)
    x = np.asarray(x, dtype=np.float32)
    # pre-transpose + fp16-cast activations on host (same rounding the
    # previous in-flight DMA cast applied, just done before the transpose)
    yts = [np.ascontiguousarray(y[b].T.astype(np.float16)) for b in range(B)]
    xts = [np.ascontiguousarray(x[b].T.astype(np.float16)) for b in range(B)]
    wqt = np.ascontiguousarray((np.asarray(Wq) * SCALE).T.astype(np.float16))
    wkt = np.ascontiguousarray(np.asarray(Wk).T.astype(np.float16))
    wvt = np.ascontiguousarray(np.asarray(Wv).T.astype(np.float16))
    bq32 = (np.asarray(bq) * SCALE).astype(np.float32)
    bk32 = np.asarray(bk, dtype=np.float32)
    bv32 = np.asarray(bv, dtype=np.float32)
    return [
        {
            "yt": yts[b],
            "xt": xts[b],
            "wqt": wqt,
            "wkt": wkt,
            "wvt": wvt,
            "bq": bq32,
            "bk": bk32,
            "bv": bv32,
        }
        for b in range(B)
    ]


def run(inputs, trace=False, trace_cores=None):
    nc = _get_nc()
    in_maps = _prep_in_maps(**inputs)
    r = run_bass_kernel_spmd(
        nc, in_maps, list(range(B)), trace=trace, trace_cores=trace_cores
    )
    out = np.stack([r.results[b]["out"] for b in range(B)], axis=0)
    return out, r


def kernel(**inputs) -> np.ndarray:
    out, _ = run(inputs, trace=False)
    return out
